# revision 5
# baseline (speedup 1.0000x reference)
"""Trainium2 Bass kernel for the 15-qubit, 4-layer variational circuit.

Problem: batch of 32 circuit evaluations; each evolves a 2^15 state through
4 layers of (RY RZ RX RZ RX per qubit + CNOT chain) and measures <Z...Z>.

Strategy (8 NeuronCores, batch-parallel, zero collectives):
  - Each core simulates 4 batch elements; the full state (4 x 32768 complex64
    as separate f32 re/im planes) lives in SBUF the whole time.
  - State layout per batch: S[p, f], p = 7 "partition qubits" (6,0,1,2,3,4,5
    MSB->LSB), f = 8 "free qubits" (7..14 MSB->LSB).
  - Per layer, two PE matmul stages, each of which applies a dense fused gate
    matrix AND transposes the layout (out = lhsT.T @ rhs with the state as the
    stationary operand):
      stage A: A = C_P @ kron(G_q for partition qubits)   (128x128 complex)
      stage B: K = C_F @ kron(G_q for free qubits)        (256x256 complex)
    where G_q = RX RZ RX RZ(x2) RY(x1) is the per-qubit fused 1q gate and
    C_P/C_F are the intra-group CNOT chains folded in host-side.
  - The one straddling CNOT(6,7) conjugated past C_F becomes
    "if q6(p): f ^= 0xFF" (free-index reversal), folded into the PSUM->SBUF
    eviction copy after stage B at zero cost. On the last layer it is skipped
    entirely (XOR by 0xFF has even parity, so the Z...Z sign is unchanged).
  - Real/imag cross terms are packed side by side in the moving operand so
    every matmul has free dim >= 256, where float32r runs at full PE rate.
  - Finale: ACT squares |amp|^2 straight out of PSUM, DVE applies the
    (-1)^popcount sign tile, ACT row-reduces via accum_out, one tiny matmul
    reduces over partitions, and a [1,4] DMA returns the 4 expectations.
"""

import sys

if "/opt/trn_rl_repo" not in sys.path:
    sys.path.append("/opt/trn_rl_repo")

import numpy as np

N_QUBITS = 15
N_LAYERS = 4
BATCH = 32
DIM = 1 << N_QUBITS
N_CORES = 8
NB = BATCH // N_CORES  # batches per core

PART_QUBITS = [6, 0, 1, 2, 3, 4, 5]       # p bit MSB->LSB (q6 = p MSB)
FREE_QUBITS = [7, 8, 9, 10, 11, 12, 13, 14]  # f bit MSB->LSB

USE_F32R = True

# ----------------------------------------------------------------- host math


def _rx(t):
    c, s = np.cos(t / 2), -1j * np.sin(t / 2)
    return np.array([[c, s], [s, c]], dtype=np.complex128)


def _ry(t):
    c, s = np.cos(t / 2), np.sin(t / 2)
    return np.array([[c, -s], [s, c]], dtype=np.complex128)


def _rz(t):
    return np.array(
        [[np.exp(-1j * t / 2), 0], [0, np.exp(1j * t / 2)]], dtype=np.complex128
    )


def _chain_perm(qubit_list, nbits, bitpos):
    """perm[old] = new index after CNOT(q, q+1) for q in qubit_list."""
    idx = np.arange(1 << nbits)
    bits = {q: (idx >> (nbits - 1 - pos)) & 1 for q, pos in bitpos.items()}
    for q in qubit_list:
        bits[q + 1] = bits[q + 1] ^ bits[q]
    new = np.zeros(1 << nbits, dtype=np.int64)
    for q, pos in bitpos.items():
        new |= bits[q] << (nbits - 1 - pos)
    return new


def _kron_list(mats):
    out = np.array([[1.0 + 0j]])
    for m in mats:
        out = np.kron(out, m)
    return out


_P_BITPOS = {q: i for i, q in enumerate(PART_QUBITS)}
_F_BITPOS = {q: i for i, q in enumerate(FREE_QUBITS)}
_PERM_P = _chain_perm(range(0, 6), 7, _P_BITPOS)
_PERM_F = _chain_perm(range(7, 14), 8, _F_BITPOS)


def _stage_matrices(x_b, thetas):
    x1 = np.arcsin(np.float64(x_b))
    x2 = np.arccos(np.float64(x_b) ** 2)
    E = _rz(x2) @ _ry(x1)
    As, Ks = [], []
    for l in range(N_LAYERS):
        G = {}
        for q in range(N_QUBITS):
            th = thetas[l, q].astype(np.float64)
            G[q] = _rx(th[2]) @ _rz(th[1]) @ _rx(th[0]) @ E
        kp = _kron_list([G[q] for q in PART_QUBITS])
        A = np.zeros_like(kp)
        A[_PERM_P, :] = kp
        kf = _kron_list([G[q] for q in FREE_QUBITS])
        K = np.zeros_like(kf)
        K[_PERM_F, :] = kf
        As.append(A)
        Ks.append(K)
    return As, Ks


def _sign_tile():
    pc = lambda v: np.array([bin(int(i)).count("1") for i in v])
    sp = 1.0 - 2.0 * (pc(np.arange(128)) % 2)
    sf = 1.0 - 2.0 * (pc(np.arange(256)) % 2)
    return (sp[:, None] * sf[None, :]).astype(np.float32)


def _host_inputs_for_core(x_core, thetas):
    """MA [NB*4, 128, 384] and MB [NB*4, 2, 128, 768] f32 for one core."""
    ma = np.zeros((NB * N_LAYERS, 128, 384), dtype=np.float32)
    mb = np.zeros((NB * N_LAYERS, 2, 128, 768), dtype=np.float32)
    for b in range(NB):
        As, Ks = _stage_matrices(x_core[b], thetas)
        for l in range(N_LAYERS):
            A = As[l]
            ArT = np.ascontiguousarray(A.real.T).astype(np.float32)
            AiT = np.ascontiguousarray(A.imag.T).astype(np.float32)
            ma[b * N_LAYERS + l] = np.concatenate([AiT, ArT, -AiT], axis=1)
            KT = Ks[l].T
            KTr = KT.real.astype(np.float32)
            KTi = KT.imag.astype(np.float32)
            for h in range(2):
                r = KTr[h * 128 : (h + 1) * 128]
                i = KTi[h * 128 : (h + 1) * 128]
                mb[b * N_LAYERS + l, h] = np.concatenate([i, r, -i], axis=1)
    return ma, mb


# -------------------------------------------------------------- device build

_CACHE = {}


def _build_module():
    import concourse.bacc as bacc
    import concourse.mybir as mybir
    import concourse.tile as tile

    f32 = mybir.dt.float32
    dtr = mybir.dt.float32r if USE_F32R else mybir.dt.float32
    Square = mybir.ActivationFunctionType.Square
    Copy = mybir.ActivationFunctionType.Copy

    nc = bacc.Bacc("TRN2", target_bir_lowering=False, debug=False)
    ma_d = nc.dram_tensor("ma", [NB * N_LAYERS, 128, 384], dtr, kind="ExternalInput")
    mb_d = nc.dram_tensor("mb", [NB * N_LAYERS, 2, 128, 768], dtr, kind="ExternalInput")
    sg_d = nc.dram_tensor("sg", [128, 256], f32, kind="ExternalInput")
    init_d = nc.dram_tensor("init", [2, 128, 256], dtr, kind="ExternalInput")
    res_d = nc.dram_tensor("res", [1, NB], f32, kind="ExternalOutput")

    with tile.TileContext(nc) as tc:
        with tc.tile_pool(name="state", bufs=1) as stp, \
             tc.tile_pool(name="xbuf", bufs=2) as xp, \
             tc.tile_pool(name="mats", bufs=3) as mp, \
             tc.tile_pool(name="fin", bufs=2) as fp, \
             tc.tile_pool(name="misc", bufs=1) as msc, \
             tc.tile_pool(name="pa", bufs=2, space="PSUM") as pa, \
             tc.tile_pool(name="pb", bufs=2, space="PSUM") as pb, \
             tc.tile_pool(name="pf", bufs=1, space="PSUM") as pf:

            S = [[stp.tile([128, 256], dtr, tag=f"S{c}{b}", name=f"S{c}{b}")
                  for c in range(2)] for b in range(NB)]
            sign = msc.tile([128, 256], f32, tag="sign")
            ones = msc.tile([128, 1], f32, tag="ones")
            rows = msc.tile([128, NB], f32, tag="rows")
            res_s = msc.tile([1, NB], f32, tag="res")

            nc.sync.dma_start(sign[:], sg_d[:])
            nc.vector.memset(ones[:], 1.0)
            nc.vector.memset(rows[:], 0.0)
            for b in range(NB):
                nc.sync.dma_start(S[b][0][:], init_d[0])
                nc.sync.dma_start(S[b][1][:], init_d[1])

            for l in range(N_LAYERS):
                for b in range(NB):
                    i_bl = b * N_LAYERS + l
                    mat_a = mp.tile([128, 384], dtr, tag="ma")
                    nc.sync.dma_start(mat_a[:], ma_d[i_bl])
                    ps_a = [pa.tile([128, 256], f32, tag=f"pa{h}", name=f"pa{h}")
                            for h in (0, 1)]
                    for h in (0, 1):
                        sl = slice(h * 128, (h + 1) * 128)
                        nc.tensor.matmul(ps_a[h][:], S[b][0][:, sl],
                                         mat_a[:, 0:256], start=True, stop=False)
                        nc.tensor.matmul(ps_a[h][:], S[b][1][:, sl],
                                         mat_a[:, 128:384], start=False, stop=True)
                    xr = xp.tile([128, 256], dtr, tag="xr")
                    xi = xp.tile([128, 256], dtr, tag="xi")
                    nc.vector.tensor_copy(xi[:, 0:128], ps_a[0][:, 0:128])
                    nc.scalar.copy(xr[:, 0:128], ps_a[0][:, 128:256])
                    nc.vector.tensor_copy(xi[:, 128:256], ps_a[1][:, 0:128])
                    nc.scalar.copy(xr[:, 128:256], ps_a[1][:, 128:256])

                    ps_b = pb.tile([128, 512], f32, tag="pb")
                    for h in (0, 1):
                        sl = slice(h * 128, (h + 1) * 128)
                        mat_b = mp.tile([128, 768], dtr, tag="mb")
                        nc.sync.dma_start(mat_b[:], mb_d[i_bl, h])
                        nc.tensor.matmul(ps_b[:], xr[:, sl],
                                         mat_b[:, 0:512],
                                         start=(h == 0), stop=False)
                        nc.tensor.matmul(ps_b[:], xi[:, sl],
                                         mat_b[:, 256:768],
                                         start=False, stop=(h == 1))
                    if l < N_LAYERS - 1:
                        # CNOT(6,7) conjugated past C_F: partitions with q6=1
                        # read the 256-wide block reversed.
                        nc.vector.tensor_copy(S[b][1][0:64, :], ps_b[0:64, 0:256])
                        nc.scalar.copy(S[b][0][0:64, :], ps_b[0:64, 256:512])
                        nc.vector.tensor_copy(S[b][1][64:128, :],
                                              ps_b[64:128, 255::-1])
                        nc.scalar.copy(S[b][0][64:128, :],
                                       ps_b[64:128, 511:255:-1])
                    else:
                        sq_r = fp.tile([128, 256], f32, tag="sqr")
                        sq_i = fp.tile([128, 256], f32, tag="sqi")
                        nc.scalar.activation(sq_r[:], ps_b[:, 256:512], Square)
                        nc.scalar.activation(sq_i[:], ps_b[:, 0:256], Square)
                        nc.vector.tensor_add(sq_r[:], sq_r[:], sq_i[:])
                        nc.vector.tensor_mul(sq_r[:], sq_r[:], sign[:])
                        nc.scalar.activation(sq_i[:], sq_r[:], Copy,
                                             accum_out=rows[:, b : b + 1])

            ps_f = pf.tile([1, NB], f32, tag="pf")
            nc.tensor.matmul(ps_f[:], ones[:].bitcast(f32), rows[:],
                             start=True, stop=True)
            nc.vector.tensor_copy(res_s[:], ps_f[:])
            nc.sync.dma_start(res_d[:], res_s[:])

    nc.compile()
    return nc


def _get_nc():
    key = "f32r" if USE_F32R else "f32"
    if key not in _CACHE:
        _CACHE[key] = _build_module()
    return _CACHE[key]


# ----------------------------------------------------------------- interface


def _run(x, thetas, trace=False):
    from concourse.bass_utils import run_bass_kernel_spmd

    x = np.asarray(x, dtype=np.float32)
    thetas = np.asarray(thetas, dtype=np.float32)
    sign = _sign_tile()
    init = np.zeros((2, 128, 256), dtype=np.float32)
    init[0, 0, 0] = 1.0
    in_maps = []
    for c in range(N_CORES):
        ma, mb = _host_inputs_for_core(x[c * NB : (c + 1) * NB], thetas)
        in_maps.append({"ma": ma, "mb": mb, "sg": sign, "init": init})
    nc = _get_nc()
    try:
        r = run_bass_kernel_spmd(nc, in_maps, core_ids=list(range(N_CORES)),
                                 trace=trace)
    except ModuleNotFoundError:
        r = run_bass_kernel_spmd(nc, in_maps, core_ids=list(range(N_CORES)),
                                 trace=False)
    out = np.concatenate([r.results[c]["res"].reshape(NB) for c in range(N_CORES)])
    return out.astype(np.float32), r


def kernel(x, thetas):
    out, _ = _run(x, thetas, trace=False)
    return out


# revision 7
# speedup vs baseline: 1.2609x; 1.2609x over previous
"""Trainium2 Bass kernel for the 15-qubit, 4-layer variational circuit.

Problem: batch of 32 circuit evaluations; each evolves a 2^15 state through
4 layers of (RY RZ RX RZ RX per qubit + CNOT chain) and measures <Z...Z>.

Strategy (8 NeuronCores, batch-parallel, zero collectives):
  - Each core simulates 4 batch elements; the full state (4 x 32768 complex64
    as separate f32 re/im planes) lives in SBUF the whole time.
  - State layout per batch: S[p, f], p = 7 "partition qubits" (6,0,1,2,3,4,5
    MSB->LSB), f = 8 "free qubits" (7..14 MSB->LSB).
  - Per layer, two PE matmul stages, each of which applies a dense fused gate
    matrix AND transposes the layout (out = lhsT.T @ rhs with the state as the
    stationary operand):
      stage A: A = C_P @ kron(G_q for partition qubits)   (128x128 complex)
      stage B: K = C_F @ kron(G_q for free qubits)        (256x256 complex)
    where G_q = RX RZ RX RZ(x2) RY(x1) is the per-qubit fused 1q gate and
    C_P/C_F are the intra-group CNOT chains folded in host-side.
  - The one straddling CNOT(6,7) conjugated past C_F becomes
    "if q6(p): f ^= 0xFF" (free-index reversal), folded into the PSUM->SBUF
    eviction copy after stage B at zero cost. On the last layer it is skipped
    entirely (XOR by 0xFF has even parity, so the Z...Z sign is unchanged).
  - Real/imag cross terms are packed side by side in the moving operand so
    every matmul has free dim >= 256, where float32r runs at full PE rate.
  - Finale: ACT squares |amp|^2 straight out of PSUM, DVE applies the
    (-1)^popcount sign tile, ACT row-reduces via accum_out, one tiny matmul
    reduces over partitions, and a [1,4] DMA returns the 4 expectations.
"""

import sys

if "/opt/trn_rl_repo" not in sys.path:
    sys.path.append("/opt/trn_rl_repo")

import numpy as np

N_QUBITS = 15
N_LAYERS = 4
BATCH = 32
DIM = 1 << N_QUBITS
N_CORES = 8
NB = BATCH // N_CORES  # batches per core

PART_QUBITS = [6, 0, 1, 2, 3, 4, 5]       # p bit MSB->LSB (q6 = p MSB)
FREE_QUBITS = [7, 8, 9, 10, 11, 12, 13, 14]  # f bit MSB->LSB

A_DT = "f32r"   # stage-A matmul dtype: f32r | bf16 | f32
B_DT = "f32r"   # stage-B matmul dtype

# ----------------------------------------------------------------- host math


def _rx(t):
    c, s = np.cos(t / 2), -1j * np.sin(t / 2)
    return np.array([[c, s], [s, c]], dtype=np.complex128)


def _ry(t):
    c, s = np.cos(t / 2), np.sin(t / 2)
    return np.array([[c, -s], [s, c]], dtype=np.complex128)


def _rz(t):
    return np.array(
        [[np.exp(-1j * t / 2), 0], [0, np.exp(1j * t / 2)]], dtype=np.complex128
    )


def _chain_perm(qubit_list, nbits, bitpos):
    """perm[old] = new index after CNOT(q, q+1) for q in qubit_list."""
    idx = np.arange(1 << nbits)
    bits = {q: (idx >> (nbits - 1 - pos)) & 1 for q, pos in bitpos.items()}
    for q in qubit_list:
        bits[q + 1] = bits[q + 1] ^ bits[q]
    new = np.zeros(1 << nbits, dtype=np.int64)
    for q, pos in bitpos.items():
        new |= bits[q] << (nbits - 1 - pos)
    return new


def _kron_list(mats):
    out = np.array([[1.0 + 0j]])
    for m in mats:
        out = np.kron(out, m)
    return out


_P_BITPOS = {q: i for i, q in enumerate(PART_QUBITS)}
_F_BITPOS = {q: i for i, q in enumerate(FREE_QUBITS)}
_PERM_P = _chain_perm(range(0, 6), 7, _P_BITPOS)
_PERM_F = _chain_perm(range(7, 14), 8, _F_BITPOS)


def _stage_matrices(x_b, thetas):
    x1 = np.arcsin(np.float64(x_b))
    x2 = np.arccos(np.float64(x_b) ** 2)
    E = _rz(x2) @ _ry(x1)
    As, Ks = [], []
    for l in range(N_LAYERS):
        G = {}
        for q in range(N_QUBITS):
            th = thetas[l, q].astype(np.float64)
            G[q] = _rx(th[2]) @ _rz(th[1]) @ _rx(th[0]) @ E
        kp = _kron_list([G[q] for q in PART_QUBITS])
        A = np.zeros_like(kp)
        A[_PERM_P, :] = kp
        kf = _kron_list([G[q] for q in FREE_QUBITS])
        K = np.zeros_like(kf)
        K[_PERM_F, :] = kf
        As.append(A)
        Ks.append(K)
    return As, Ks


def _sign_tile():
    pc = lambda v: np.array([bin(int(i)).count("1") for i in v])
    sp = 1.0 - 2.0 * (pc(np.arange(128)) % 2)
    sf = 1.0 - 2.0 * (pc(np.arange(256)) % 2)
    return (sp[:, None] * sf[None, :]).astype(np.float32)


def _host_inputs_for_core(x_core, thetas):
    """MA [NB*4, 128, 384] and MB [NB*4, 2, 128, 768] f32 for one core."""
    ma = np.zeros((NB * N_LAYERS, 128, 256), dtype=np.float32)
    mb = np.zeros((NB * N_LAYERS, 128, 1024), dtype=np.float32)
    for b in range(NB):
        As, Ks = _stage_matrices(x_core[b], thetas)
        for l in range(N_LAYERS):
            A = As[l]
            ArT = np.ascontiguousarray(A.real.T).astype(np.float32)
            AiT = np.ascontiguousarray(A.imag.T).astype(np.float32)
            ma[b * N_LAYERS + l] = np.concatenate([AiT, ArT], axis=1)
            KT = Ks[l].T
            KTr = KT.real.astype(np.float32)
            KTi = KT.imag.astype(np.float32)
            mb[b * N_LAYERS + l] = np.concatenate(
                [KTi[0:128], KTr[0:128], KTi[128:256], KTr[128:256]], axis=1)
    return ma, mb


# -------------------------------------------------------------- device build

_CACHE = {}


def _build_module():
    import concourse.bacc as bacc
    import concourse.mybir as mybir
    import concourse.tile as tile

    f32 = mybir.dt.float32
    dts = {"f32r": mybir.dt.float32r, "bf16": mybir.dt.bfloat16,
           "f16": mybir.dt.float16, "f32": mybir.dt.float32}
    dta = dts[A_DT]
    dtb = dts[B_DT]
    Square = mybir.ActivationFunctionType.Square
    Copy = mybir.ActivationFunctionType.Copy

    nc = bacc.Bacc("TRN2", target_bir_lowering=False, debug=False)
    ma_d = nc.dram_tensor("ma", [NB * N_LAYERS, 128, 256], dta, kind="ExternalInput")
    mb_d = nc.dram_tensor("mb", [NB * N_LAYERS, 128, 1024], dtb, kind="ExternalInput")
    sg_d = nc.dram_tensor("sg", [128, 256], f32, kind="ExternalInput")
    one_d = nc.dram_tensor("one", [1, NB], dta, kind="ExternalInput")
    res_d = nc.dram_tensor("res", [1, NB], f32, kind="ExternalOutput")

    with tile.TileContext(nc) as tc:
        with tc.tile_pool(name="state", bufs=1) as stp, \
             tc.tile_pool(name="xbuf", bufs=3) as xp, \
             tc.tile_pool(name="mats", bufs=3) as mp, \
             tc.tile_pool(name="fin", bufs=2) as fp, \
             tc.tile_pool(name="misc", bufs=1) as msc, \
             tc.tile_pool(name="pa", bufs=2, space="PSUM") as pa, \
             tc.tile_pool(name="pb", bufs=2, space="PSUM") as pb, \
             tc.tile_pool(name="pf", bufs=1, space="PSUM") as pf:

            # state: 4 batches side by side, re/im planes
            s_r = stp.tile([128, 256 * NB], dta, tag="sr", name="sr")
            s_i = stp.tile([128, 256 * NB], dta, tag="si", name="si")
            sign = msc.tile([128, 256], f32, tag="sign")
            ones = msc.tile([128, 1], f32, tag="ones")
            rows = msc.tile([128, NB], f32, tag="rows")
            res_s = msc.tile([1, NB], f32, tag="res")
            ztmp = msc.tile([128, 256 * NB], f32, tag="ztmp")

            nc.sync.dma_start(sign[:], sg_d[:])
            nc.vector.memset(ones[:], 1.0)
            nc.vector.memset(rows[:], 0.0)
            nc.vector.memset(ztmp[:], 0.0)
            nc.vector.tensor_copy(s_r[:], ztmp[:])
            nc.scalar.copy(s_i[:], ztmp[:])
            # unit impulse at (p=0, f=0) of each batch block
            nc.sync.dma_start(s_r[0:1, 0 : 256 * NB : 256], one_d[:])

            for l in range(N_LAYERS):
                for b in range(NB):
                    i_bl = b * N_LAYERS + l
                    # [AiT | ArT | -AiT]
                    mta = mp.tile([128, 384], dta, tag="ma")
                    nc.sync.dma_start(mta[:, 0:256], ma_d[i_bl])
                    nc.vector.tensor_scalar_mul(mta[:, 256:384],
                                                mta[:, 0:128], -1.0)
                    # [Ki0 | Kr0 | Ki1 | Kr1 | -Ki0 | -Ki1]
                    mtb = mp.tile([128, 1536], dtb, tag="mb")
                    nc.sync.dma_start(mtb[:, 0:1024], mb_d[i_bl])
                    nc.vector.tensor_scalar_mul(mtb[:, 1024:1280],
                                                mtb[:, 0:256], -1.0)
                    nc.scalar.activation(mtb[:, 1280:1536], mtb[:, 512:768],
                                         Copy, scale=-1.0)

                    sb = slice(b * 256, (b + 1) * 256)
                    ps_a = pa.tile([128, 512], f32, tag="pa")
                    for h in (0, 1):
                        sl = slice(b * 256 + h * 128, b * 256 + (h + 1) * 128)
                        po = slice(h * 256, (h + 1) * 256)
                        nc.tensor.matmul(ps_a[:, po], s_r[:, sl],
                                         mta[:, 0:256],
                                         start=(h == 0), stop=False)
                        nc.tensor.matmul(ps_a[:, po], s_i[:, sl],
                                         mta[:, 128:384],
                                         start=False, stop=(h == 1))
                    xr = xp.tile([128, 256], dtb, tag="xr")
                    xi = xp.tile([128, 256], dtb, tag="xi")
                    pav = ps_a[:].rearrange("u (h c p) -> u h c p", c=2, p=128)
                    nc.vector.tensor_copy(
                        xi[:].rearrange("u (h p) -> u h p", p=128),
                        pav[:, :, 0, :])
                    nc.scalar.copy(
                        xr[:].rearrange("u (h p) -> u h p", p=128),
                        pav[:, :, 1, :])

                    ps_b = pb.tile([128, 512], f32, tag="pb")
                    mtb6 = mtb[:].rearrange("u (a v) -> u a v", v=256)
                    rhs_xi = [mtb6[:, 1::3, :], mtb6[:, 3::2, :]]
                    for h in (0, 1):
                        hb = slice(h * 128, (h + 1) * 128)
                        nc.tensor.matmul(ps_b[:], xr[:, hb],
                                         mtb[:, h * 512 : (h + 1) * 512],
                                         start=(h == 0), stop=False)
                        nc.tensor.matmul(ps_b[:], xi[:, hb], rhs_xi[h],
                                         start=False, stop=(h == 1))
                    if l < N_LAYERS - 1:
                        nc.vector.tensor_copy(s_i[0:64, sb], ps_b[0:64, 0:256])
                        nc.scalar.copy(s_r[0:64, sb], ps_b[0:64, 256:512])
                        nc.vector.tensor_copy(s_i[64:128, sb],
                                              ps_b[64:128, 255::-1])
                        nc.scalar.copy(s_r[64:128, sb],
                                       ps_b[64:128, 511:255:-1])
                    else:
                        sq_r = fp.tile([128, 256], f32, tag="sqr")
                        sq_i = fp.tile([128, 256], f32, tag="sqi")
                        nc.scalar.activation(sq_r[:], ps_b[:, 256:512], Square)
                        nc.scalar.activation(sq_i[:], ps_b[:, 0:256], Square)
                        nc.vector.tensor_add(sq_r[:], sq_r[:], sq_i[:])
                        nc.vector.tensor_mul(sq_r[:], sq_r[:], sign[:])
                        nc.scalar.activation(sq_i[:], sq_r[:], Copy,
                                             accum_out=rows[:, b : b + 1])

            ps_f = pf.tile([1, NB], f32, tag="pf")
            nc.tensor.matmul(ps_f[:], ones[:], rows[:], start=True, stop=True)
            nc.vector.tensor_copy(res_s[:], ps_f[:])
            nc.sync.dma_start(res_d[:], res_s[:])

    nc.compile()
    return nc


def _get_nc():
    key = A_DT + B_DT
    if key not in _CACHE:
        _CACHE[key] = _build_module()
    return _CACHE[key]


# ----------------------------------------------------------------- interface


def _run(x, thetas, trace=False):
    from concourse.bass_utils import run_bass_kernel_spmd

    import ml_dtypes

    x = np.asarray(x, dtype=np.float32)
    thetas = np.asarray(thetas, dtype=np.float32)
    np_map = {"bf16": ml_dtypes.bfloat16, "f16": np.float16,
              "f32r": np.float32, "f32": np.float32}
    np_a = np_map[A_DT]
    np_b = np_map[B_DT]
    sign = _sign_tile()
    one = np.ones((1, NB), dtype=np_a)
    in_maps = []
    for c in range(N_CORES):
        ma, mb = _host_inputs_for_core(x[c * NB : (c + 1) * NB], thetas)
        in_maps.append({"ma": ma.astype(np_a), "mb": mb.astype(np_b),
                        "sg": sign, "one": one})
    nc = _get_nc()
    try:
        r = run_bass_kernel_spmd(nc, in_maps, core_ids=list(range(N_CORES)),
                                 trace=trace)
    except ModuleNotFoundError:
        r = run_bass_kernel_spmd(nc, in_maps, core_ids=list(range(N_CORES)),
                                 trace=False)
    out = np.concatenate([r.results[c]["res"].reshape(NB) for c in range(N_CORES)])
    return out.astype(np.float32), r


def kernel(x, thetas):
    out, _ = _run(x, thetas, trace=False)
    return out
